# revision 43
# baseline (speedup 1.0000x reference)
"""Multi-head attention (B=4, T=2048, D=1024, H=16, causal) on 8 trn2 cores.

Sharding: 2 heads per core (tensor-parallel on H). Each core computes
q/k/v projections for its 128-row slice of Wq/Wk/Wv over all tokens,
causal attention for its 2 heads over all 4 batches, and a partial
o_proj contribution y_c = A_c @ Wo[:, slice].T.  The host sums the 8
partial outputs (the "all-reduce after o_proj" of the sharding hint).

Layout strategy: activations live transposed ([feature, token]) on
device so the matmul contraction dim is always the partition dim:
  qT/kT/vT [128=2*64, T]  <- W-slice-T tiles @ xT chunks
  scoresT  [128 k-tok, 512 q-tok] <- kT-tile.T @ qT   (per head)
  causal:  diagonal k-tiles trim scores/exp/attn@v to the valid q-span
           [off, QC) (no masked-region compute, no ep memsets); the
           intra-tile triangle is a 0/1 band multiply.
  softmax: no max-subtraction (logits are O(+-8); exp is safe in f32);
           exp on ACT; row sums come free as a column of ones appended
           to v; normalization deferred to after attn@v: broadcast the
           denominators across partitions with K=1 PE outer products,
           then reciprocal+scale in column HALVES so the first o_proj
           piece (needs only aT[:, 0:128]) starts after half the
           reciprocal latency.
  aT       [65, 512] PSUM accum over k-tiles (row 64 = softmax denom)
  y        [tok, D] bf16 partial via lhsT=aT tiles (halves the output
           DMA; the host sums the 8 partials in f32).
Matmuls run in bf16 (1 cyc/row on the PE; f32r measured 2 cyc/row).

DMA layouts are pre-shuffled on the host so every device DMA is
contiguous per partition (4KB+ descriptors): xT arrives as
[128, B, 8, T], weights as [128, 8, 128], y leaves as
[128, chunk, 4, D].  vT is bf16 end-to-end (bf16 PE transposes at
1 cyc/row, 2x DVE copy rate).  o_proj evacuation alternates
DVE/ACT and PSUM banks so the drain is double-buffered.
"""

import sys

sys.path.insert(0, "/opt/trn_rl_repo")

import ml_dtypes
import numpy as np

import concourse.bass as bass
import concourse.mybir as mybir
import concourse.tile as tile
from concourse.masks import make_identity

F32 = mybir.dt.float32
U32 = mybir.dt.uint32
F32R = mybir.dt.float32r
BF16 = mybir.dt.bfloat16
EXP = mybir.ActivationFunctionType.Exp

B, T, D, H = 4, 2048, 1024, 16
DH = D // H
NCORES = 8
HPC = H // NCORES          # heads per core (2)
HS = HPC * DH              # head-slice width per core (128)
QC = 512                   # q-tokens per chunk (PSUM free-dim limit, f32)
KT = 128                   # k-tokens per tile (partition dim)

_wsplit_n = [0]


def split_embedded_waits(nc):
    """Move embedded on_wait conditions into standalone EventSemaphore
    instructions.  The walrus build in this container rejects embedded
    sync waits on compute instruction structs ("Too many sync wait
    commands"); raw-bass-style standalone waits compile fine."""
    moved = 0
    for func in nc.m.functions:
        for blk in func.blocks:
            insts = list(blk.instructions)
            out = []
            changed = False
            for ins in insts:
                si = ins.sync_info
                waits = list(si.on_wait) if (si is not None and si.on_wait) else []
                limit = 1 if ins.opcode in ("EventSemaphore", "Drain") else 0
                if len(waits) > limit:
                    head = waits[:-limit] if limit else waits
                    tail = waits[-limit:] if limit else []
                    for w in head:
                        _wsplit_n[0] += 1
                        out.append(
                            mybir.InstEventSemaphore(
                                name=f"I-wsplit-{_wsplit_n[0]}",
                                engine=ins.engine,
                                sync_info=mybir.SyncInfo(on_wait=[w], on_update=[]),
                            )
                        )
                        moved += 1
                    ins.sync_info = mybir.SyncInfo(
                        on_wait=list(tail),
                        on_update=list(si.on_update) if si.on_update else [],
                    )
                    changed = True
                out.append(ins)
            if changed:
                blk.instructions = out
    return moved


def build_nc(nb=B, t=T, split_waits=True):
    """Build the per-core Bass program. nb/t shrinkable for simulation.
    split_waits must be True for hardware (walrus); False for CoreSim."""
    nqc = t // QC            # q-chunks per batch
    nkt = t // KT            # k-tiles per batch
    ntok = nb * t
    nd = D // 128            # 8 d-in tiles

    nc = bass.Bass("TRN2", target_bir_lowering=False)

    # pre-shuffled host layouts: every DMA is contiguous per partition
    xT_d = nc.dram_tensor("xT", [128, nb, nqc, nd, QC], BF16, kind="ExternalInput")
    wq_d = nc.dram_tensor("wq", [128, nd, HS], BF16, kind="ExternalInput")
    wk_d = nc.dram_tensor("wk", [128, nd, HS], BF16, kind="ExternalInput")
    wv_d = nc.dram_tensor("wv", [128, nd, HS], BF16, kind="ExternalInput")
    wo_d = nc.dram_tensor("wo", [HS, D], BF16, kind="ExternalInput")
    tri_d = nc.dram_tensor("tri", [KT, KT], BF16, kind="ExternalInput")
    y_d = nc.dram_tensor("y", [128, nb * nqc, QC // 128, D], BF16,
                         kind="ExternalOutput")

    with tile.TileContext(nc) as tc:
        with (
            tc.tile_pool(name="wpool", bufs=1) as wpool,
            tc.tile_pool(name="xin", bufs=2) as xin,
            tc.tile_pool(name="qkv", bufs=2) as qkvp,
            tc.tile_pool(name="vaug", bufs=2) as vaugp,
            tc.tile_pool(name="exps", bufs=4) as expp,
            tc.tile_pool(name="misc", bufs=2) as miscp,
            tc.tile_pool(name="yout", bufs=2) as youtp,
            tc.tile_pool(name="psc", bufs=2, space="PSUM") as psc,
            tc.tile_pool(name="pa", bufs=1, space="PSUM") as pa,
            tc.tile_pool(name="pm", bufs=2, space="PSUM") as pm,
        ):
            # resident weights (issue order = DMA arrival priority: the
            # first QKV matmul needs wq + the first x slice)
            wq_sb = wpool.tile([128, nd, HS], BF16, tag="wq")
            wk_sb = wpool.tile([128, nd, HS], BF16, tag="wk")
            wv_sb = wpool.tile([128, nd, HS], BF16, tag="wv")
            nc.sync.dma_start(wq_sb[:], wq_d[:])

            def load_xt(b):
                xt = xin.tile([128, nqc, nd, QC], BF16, tag="xt")
                nc.sync.dma_start(xt[:], xT_d[:, b])
                return xt

            def load_xt_split(b, c, xt=None):
                # one q-chunk of x: 8KB contiguous per partition
                if xt is None:
                    xt = xin.tile([128, nqc, nd, QC], BF16, tag="xt")
                nc.sync.dma_start(xt[:, c], xT_d[:, b, c])
                return xt

            # chunk 0 of batch 0 needs wq + x[0:512] + wk + wv, in that
            # order; the rest of batch 0's x can trail behind.
            xt_next = load_xt_split(0, 0)
            nc.sync.dma_start(wk_sb[:], wk_d[:])
            nc.sync.dma_start(wv_sb[:], wv_d[:])
            for s in range(1, nqc):
                load_xt_split(0, s, xt_next)
            wo_sb = wpool.tile([HS, D], BF16, tag="wo")
            nc.sync.dma_start(wo_sb[:], wo_d[:])
            tri_sb = wpool.tile([KT, KT], BF16, tag="tri")
            nc.sync.dma_start(tri_sb[:], tri_d[:])
            # constants: f32 masters, rounded into matmul dtypes via copies
            ones_f = wpool.tile([128, max(nkt, 128)], F32, tag="ones_f")
            nc.vector.memset(ones_f[:], 1.0)
            zeros_f = wpool.tile([1, 64], F32, tag="zeros_f")
            nc.vector.memset(zeros_f[:], 0.0)
            # head-expander rows: e0row = [1]*64+[0]*64, e1row = [0]*64+[1]*64
            e0row = wpool.tile([1, 128], F32R, tag="e0row")
            e1row = wpool.tile([1, 128], F32R, tag="e1row")
            nc.vector.tensor_copy(e0row[:, 0:64], ones_f[0:1, 0:64])
            nc.vector.tensor_copy(e0row[:, 64:128], zeros_f[:])
            nc.vector.tensor_copy(e1row[:, 0:64], zeros_f[:])
            nc.vector.tensor_copy(e1row[:, 64:128], ones_f[0:1, 0:64])
            ident = wpool.tile([128, 128], BF16, tag="ident")
            make_identity(nc, ident[:])
            ck = wpool.tile([128, 512], U32, tag="ck")
            nc.vector.memset(ck[:], 0x7EF127EA)
            # touch the ACT engine once now: walrus inserts the ~2.7us
            # activation-table load before the first ACTIVATE, and this
            # hides it under the startup DMA wait
            nc.scalar.copy(zeros_f[:], zeros_f[:])

            class OproJob:
                """Deferred o_proj for one 512-token chunk: 8 matmul+copy
                pieces stepped one at a time between k-iterations, then one
                fused DMA of the [512, D] result.  Pieces alternate PSUM
                banks (tags m/rb) and evacuation engines (DVE/ACT) so the
                matmul->copy chain is double-buffered even at drains."""

                def __init__(self, ci, aT, final=False):
                    self.ci, self.aT, self.final = ci, aT, final
                    self.ysb = youtp.tile([128, QC // 128, D], BF16, tag="ysb")
                    self.pieces = [
                        (tt, do)
                        for tt in range(QC // 128)
                        for do in range(D // 512)
                    ]
                    self.i = 0

                def step(self):
                    tt, do = self.pieces[self.i]
                    self.i += 1
                    if self.final and self.i % 2 == 0:
                        # the very last job has no following k-loop to
                        # drip through: double-buffer its drain via the
                        # psc banks (free once the last exps have run;
                        # the pm "rb" bank is NOT safe here -- the NR
                        # reciprocal still reads the denominators)
                        yp = psc.tile([128, 2, QC], F32, tag="sc", name="ypf")[:, 0, :]
                    else:
                        yp = pm.tile([128, 512], F32, tag="m", bufs=1,
                                     name="yp")
                    nc.tensor.matmul(
                        yp[:],
                        self.aT[:, 128 * tt : 128 * (tt + 1)],
                        wo_sb[:, 512 * do : 512 * (do + 1)],
                        start=True,
                        stop=True,
                    )
                    dst = self.ysb[:, tt, 512 * do : 512 * (do + 1)]
                    if self.final:
                        # ACT is idle in the endgame; keep the DVE queue
                        # clear for the half-1 normalization chain
                        nc.scalar.copy(dst, yp[:])
                    else:
                        nc.vector.tensor_copy(dst, yp[:])
                    if self.final and self.i % 2 == 0:
                        # nothing left to overlap the last DMA with: ship
                        # each finished 128-token block immediately
                        nc.sync.dma_start(
                            y_d[:, self.ci, tt, :], self.ysb[:, tt, :]
                        )
                    elif not self.final and self.i == len(self.pieces):
                        nc.sync.dma_start(y_d[:, self.ci, :, :], self.ysb[:])
                    return self.i < len(self.pieces)

            ojob = [None]

            # kT0z/kT1z zero halves never change: single-buffered tiles,
            # memset once (batches are strictly sequential on this layout)
            kT0z = qkvp.tile([128, t], BF16, tag="kT0z", bufs=1, name="kT0z")
            kT1z = qkvp.tile([128, t], BF16, tag="kT1z", bufs=1, name="kT1z")
            nc.vector.memset(kT0z[64:128, :], 0.0)
            nc.vector.memset(kT1z[0:64, :], 0.0)

            qnext = [None]  # next batch's qT, chunk 0 pre-projected
            for b in range(nb):
                xt_b = xt_next
                # ---- q/k/v projections for batch b ----
                # kT is stored zero-padded per head (kT0z rows 0:64 = head0,
                # rows 64:128 = 0; kT1z the reverse) so the scores matmul can
                # stream the full 128-partition qT at full SBUF rate.
                if qnext[0] is not None:
                    qT, q_pre = qnext[0], True
                    qnext[0] = None
                else:
                    qT, q_pre = qkvp.tile([128, t], BF16, tag="qT", name="qT"), False
                vT = qkvp.tile([128, t], BF16, tag="vT")
                for ch in range(nqc):
                    cs = slice(QC * ch, QC * (ch + 1))
                    for wi, ws in enumerate((wq_sb, wk_sb, wv_sb)):
                        if ch == 0 and wi == 0 and q_pre:
                            continue  # hoisted into the previous batch tail
                        # the first projection of a batch runs on the free
                        # pm bank: at a batch boundary both psc tiles are
                        # still being read by the previous batch's tail
                        # exps, which would idle the PE (and HAM-throttle
                        # it); by the time the second projection starts
                        # the first exp has freed its psc buffer.
                        if ch == 0 and wi == 0:
                            ps = pm.tile([128, QC], F32, tag="m", bufs=1,
                                         name="ps")[:, :]
                        else:
                            ps = psc.tile([128, 2, QC], F32, tag="sc", name="psq")[:, 0, :]
                        for kd in range(nd):
                            nc.tensor.matmul(
                                ps,
                                ws[:, kd, :],
                                xt_b[:, ch, kd, :],
                                start=(kd == 0),
                                stop=(kd == nd - 1),
                            )
                        # evacuate on the Act engine: it is idle during
                        # the qkv phase while DVE is the congested queue
                        if wi == 0:
                            nc.scalar.copy(qT[:, cs], ps)
                        elif wi == 1:
                            nc.scalar.copy(kT0z[0:64, cs], ps[0:64, :])
                            nc.scalar.copy(kT1z[64:128, cs], ps[64:128, :])
                        else:
                            nc.scalar.copy(vT[:, cs], ps[:, :])

                if b + 1 < nb:
                    xt_next = load_xt(b + 1)

                # ---- transpose v into [k-tok, dh(+ones)] tiles ----
                v0 = vaugp.tile([128, nkt, DH + 1], BF16, tag="v0")
                v1 = vaugp.tile([128, nkt, DH + 1], BF16, tag="v1")
                nc.vector.tensor_copy(v0[:, :, DH : DH + 1], ones_f[:, 0:nkt])
                nc.vector.tensor_copy(v1[:, :, DH : DH + 1], ones_f[:, 0:nkt])
                for kt0 in range(0, nkt, 4):
                    # four transposes back-to-back into one PSUM tile, then
                    # two strided copies: avoids the per-tile PE<->DVE
                    # ping-pong through the single rb bank
                    ng = min(4, nkt - kt0)
                    tp = pm.tile([128, 512], BF16, tag="rb", bufs=1, name="tp")
                    for j in range(ng):
                        kt = kt0 + j
                        nc.tensor.transpose(
                            tp[:, 128 * j : 128 * (j + 1)],
                            vT[:, KT * kt : KT * (kt + 1)],
                            ident[:],
                        )
                    tpv = tp.rearrange("p (g c) -> p g c", c=128)
                    nc.vector.tensor_copy(
                        v0[:, kt0 : kt0 + ng, 0:DH], tpv[:, 0:ng, 0:DH]
                    )
                    nc.vector.tensor_copy(
                        v1[:, kt0 : kt0 + ng, 0:DH], tpv[:, 0:ng, DH : 2 * DH]
                    )

                # ---- attention + o_proj per q-chunk ----
                # k-loop emitted software-pipelined (scores two steps ahead
                # of attn@v); the previous chunk's o_proj matmuls and output
                # copies are drip-fed between k-iterations so the PE never
                # sits in a blocked o_proj stretch, and normalization uses a
                # magic-seed Newton-Raphson reciprocal on DVE.
                carried = [None]  # pre-emitted scores for (qc+1, kt=0)
                for qc in range(nqc):
                    q0 = QC * qc
                    apair = pa.tile([DH + 1, 2, QC], F32, tag="apair")
                    hi = qc * (QC // KT) + (QC // KT)  # causal: k-tiles 0..hi-1

                    def emit_scores(kt, sq0=None):
                        sq0 = q0 if sq0 is None else sq0
                        o = max(KT * kt - sq0, 0)
                        scp = psc.tile([128, 2, QC], F32, tag="sc")
                        nc.tensor.matmul(
                            scp[:, 0, o:QC],
                            kT0z[:, KT * kt : KT * (kt + 1)],
                            qT[:, sq0 + o : sq0 + QC],
                            start=True,
                            stop=True,
                        )
                        nc.tensor.matmul(
                            scp[:, 1, o:QC],
                            kT1z[:, KT * kt : KT * (kt + 1)],
                            qT[:, sq0 + o : sq0 + QC],
                            start=True,
                            stop=True,
                        )
                        return scp

                    def emit_tail(kt, scp, tri_eng=None):
                        ep = expp.tile([128, 2, QC], BF16, tag="ep")
                        off = KT * kt - q0
                        o = max(off, 0)
                        nc.scalar.activation(ep[:, :, o:QC], scp[:, :, o:QC], EXP)
                        if off >= 0:
                            # diagonal tile: apply the 0/1 band (no memset
                            # needed -- attn@v only reads the [o:QC] span)
                            for h in (0, 1):
                                (tri_eng or nc.vector).tensor_mul(
                                    ep[:, h, o : o + KT],
                                    ep[:, h, o : o + KT],
                                    tri_sb[:],
                                )
                        for h, vh in ((0, v0), (1, v1)):
                            nc.tensor.matmul(
                                apair[:, h, o:QC],
                                vh[:, kt, :],
                                ep[:, h, o:QC],
                                start=(kt == 0),
                                stop=(kt == hi - 1),
                                skip_group_check=True,
                            )

                    if carried[0] is not None:
                        pend = [carried[0]]
                        carried[0] = None
                    else:
                        pend = [emit_scores(0)]
                    if hi > 1:
                        pend.append(emit_scores(1))
                    for kt in range(2, hi):
                        emit_tail(kt - 2, pend.pop(0))
                        pend.append(emit_scores(kt))
                        # defer the first o_proj drip one iteration: at
                        # kt=2 the piece can still be waiting on the
                        # previous chunk's aT scale, stalling the PE FIFO
                        if kt >= 3 and ojob[0] is not None:
                            if not ojob[0].step():
                                ojob[0] = None
                    final = b == nb - 1 and qc == nqc - 1
                    if final:
                        # ===== custom endgame for the very last chunk =====
                        # There is no later work to hide the normalization
                        # chain behind, so exploit causality: the last two
                        # (diagonal) k-tiles only touch apair columns
                        # [256:512], so columns [0:256] are already final.
                        # Evacuate and normalize half 0 on DVE *while* the
                        # PE runs the last two tails, then drain o_proj
                        # pieces per half.  Tri-masks go to the idle GpSimd
                        # so they don't block the DVE chain (FIFO queues).
                        H1 = QC // 2
                        sums01 = miscp.tile([1, 2, QC], F32R, tag="sums01")
                        aT = qkvp.tile([128, QC], BF16, tag="aT", bufs=3)
                        nc.vector.tensor_copy(
                            sums01[:, :, 0:H1], apair[DH : DH + 1, :, 0:H1]
                        )
                        nc.vector.tensor_copy(
                            aT[0:DH, 0:H1], apair[0:DH, 0, 0:H1]
                        )
                        nc.vector.tensor_copy(
                            aT[DH : 2 * DH, 0:H1], apair[0:DH, 1, 0:H1]
                        )
                        for j, scp in enumerate(pend):
                            emit_tail(
                                hi - len(pend) + j, scp, tri_eng=nc.gpsimd
                            )
                        rb = pm.tile([128, 512], F32, tag="rb", bufs=1)
                        rcp = miscp.tile([128, QC], F32, tag="rcp")
                        tnr = miscp.tile([128, QC], F32, tag="tnr")
                        rb_f = rb[:, 0:QC]
                        job = OproJob(b * nqc + qc, aT, final=True)

                        def warm():
                            # no-op weight load: keeps the PE active
                            # through the endgame's short waits so HAM
                            # doesn't down-clock the whole tail
                            nc.tensor.ldweights(wo_sb[:, 0:128])

                        for half in range(2):
                            hs = slice(half * H1, (half + 1) * H1)
                            if half == 1:
                                nc.vector.tensor_copy(
                                    sums01[:, :, hs],
                                    apair[DH : DH + 1, :, hs],
                                )
                                nc.vector.tensor_copy(
                                    aT[0:DH, hs], apair[0:DH, 0, hs]
                                )
                                nc.vector.tensor_copy(
                                    aT[DH : 2 * DH, hs], apair[0:DH, 1, hs]
                                )
                            nc.tensor.matmul(
                                rb[:, hs], e0row[:], sums01[:, 0, hs],
                                start=True, stop=False, skip_group_check=True,
                            )
                            nc.tensor.matmul(
                                rb[:, hs], e1row[:], sums01[:, 1, hs],
                                start=False, stop=True, skip_group_check=True,
                            )
                            nc.vector.tensor_tensor(
                                rcp[:, hs].bitcast(U32),
                                ck[:, hs],
                                rb_f[:, hs].bitcast(U32),
                                mybir.AluOpType.subtract,
                            )
                            nc.vector.tensor_mul(
                                tnr[:, hs], rb_f[:, hs], rcp[:, hs]
                            )
                            nc.vector.scalar_tensor_tensor(
                                rcp[:, hs], tnr[:, hs], 2.0, rcp[:, hs],
                                mybir.AluOpType.subtract, mybir.AluOpType.mult,
                            )
                            nc.vector.tensor_mul(
                                aT[:, hs], aT[:, hs], rcp[:, hs]
                            )
                            warm()
                            for _ in range(4):
                                job.step()
                                warm()
                        continue
                    for j, scp in enumerate(pend):
                        emit_tail(hi - len(pend) + j, scp)
                    # bridge the chunk boundary: pre-emit the next chunk's
                    # first scores pair while the norm copies drain
                    if qc + 1 < nqc:
                        carried[0] = emit_scores(0, sq0=QC * (qc + 1))
                    while ojob[0] is not None:
                        if not ojob[0].step():
                            ojob[0] = None

                    if qc == nqc - 1 and b + 1 < nb:
                        # batch boundary: fill the PE during this chunk's
                        # normalization with the NEXT batch's first
                        # q-projection (pm bank, x already resident)
                        qTn = qkvp.tile([128, t], BF16, tag="qT", name="qTn")
                        psn = pm.tile([128, QC], F32, tag="m", bufs=1,
                                      name="psn")
                        for kd in range(nd):
                            nc.tensor.matmul(
                                psn[:, :],
                                wq_sb[:, kd, :],
                                xt_next[:, 0, kd, :],
                                start=(kd == 0),
                                stop=(kd == nd - 1),
                            )
                        nc.scalar.copy(qTn[:, 0:QC], psn[:, :])
                        qnext[0] = qTn

                    # free apair fast: pull out the two heads + denominators
                    sums01 = miscp.tile([1, 2, QC], F32R, tag="sums01")
                    nc.vector.tensor_copy(sums01[:], apair[DH : DH + 1, :, :])
                    aT = qkvp.tile([128, QC], BF16, tag="aT", bufs=3)
                    nc.scalar.copy(aT[0:DH, :], apair[0:DH, 0, :])
                    nc.scalar.copy(aT[DH : 2 * DH, :], apair[0:DH, 1, :])
                    # normalization, fully pipelined by column halves: copy
                    # the denominator row out (lane-serial, the long pole),
                    # broadcast it across partitions (K=1 PE outer
                    # products), 1/s via magic-seed + one Newton-Raphson
                    # pass on DVE ([s_bits XOR ~0] + [K+1] seed in one
                    # fused op; the (t-2)*r0 combine yields -1/s, absorbed
                    # by staging -Wo), then scale aT.  Doing all five steps
                    # per half instead of splitting only the NR shortens
                    # the last-attnv -> first-o_proj critical path by ~1us.
                    rb = pm.tile([128, 512], F32, tag="rb", bufs=1)
                    nc.tensor.matmul(
                        rb[:, 0:QC], e0row[:], sums01[:, 0, :],
                        start=True, stop=False, skip_group_check=True,
                    )
                    nc.tensor.matmul(
                        rb[:, 0:QC], e1row[:], sums01[:, 1, :],
                        start=False, stop=True, skip_group_check=True,
                    )
                    rcp = miscp.tile([128, QC], F32, tag="rcp")
                    tnr = miscp.tile([128, QC], F32, tag="tnr")
                    rb_f = rb[:, 0:QC]
                    for half in range(2):
                        hs = slice(half * (QC // 2), (half + 1) * (QC // 2))
                        nc.vector.tensor_tensor(
                            rcp[:, hs].bitcast(U32),
                            ck[:, hs],
                            rb_f[:, hs].bitcast(U32),
                            mybir.AluOpType.subtract,
                        )
                        nc.vector.tensor_mul(tnr[:, hs], rb_f[:, hs], rcp[:, hs])
                        nc.vector.scalar_tensor_tensor(
                            rcp[:, hs], tnr[:, hs], 2.0, rcp[:, hs],
                            mybir.AluOpType.subtract, mybir.AluOpType.mult,
                        )
                        nc.vector.tensor_mul(aT[:, hs], aT[:, hs], rcp[:, hs])
                    ojob[0] = OproJob(b * nqc + qc, aT)
            while ojob[0] is not None:
                if not ojob[0].step():
                    ojob[0] = None

    if split_waits:
        split_embedded_waits(nc)
    return nc


def make_tri():
    tri = np.zeros((KT, KT), np.float32)
    j = np.arange(KT)[None, :]
    k = np.arange(KT)[:, None]
    tri[j >= k] = 1.0
    return tri.astype(ml_dtypes.bfloat16)


def shuffle_w(w):
    # [D, HS] -> [128, nd, HS]: row a*128+p lands at [p, a, :]
    return np.ascontiguousarray(
        w.reshape(D // 128, 128, HS).transpose(1, 0, 2)
    )


def make_in_maps(x, Wq, Wk, Wv, Wo):
    bf = ml_dtypes.bfloat16
    # x [B, T, D] -> [128, B, nqc, 8, QC]:
    # xbuf[p, b, c, a, tc] = x[b, c*QC+tc, a*128+p]  (chunk-contiguous:
    # each q-chunk is one 8KB run per partition)
    xT = np.ascontiguousarray(
        x.reshape(B, T // QC, QC, D // 128, 128).transpose(4, 0, 1, 3, 2)
    ).astype(bf)
    tri = make_tri()
    scale = np.float32(1.0 / np.sqrt(DH))
    in_maps = []
    for c in range(NCORES):
        hs = slice(HS * c, HS * (c + 1))
        in_maps.append(
            {
                "xT": xT,
                "wq": shuffle_w((Wq[hs, :] * scale).T.astype(bf)),
                "wk": shuffle_w(Wk[hs, :].T.astype(bf)),
                "wv": shuffle_w(Wv[hs, :].T.astype(bf)),
                "wo": np.ascontiguousarray(-Wo[:, hs].T).astype(bf),
                "tri": tri,
            }
        )
    return in_maps


def unshuffle_y(y):
    # y [128, nchunks, 4, D] -> [ntok, D]: token c*512 + a*128 + p
    nch = y.shape[1]
    return y.transpose(1, 2, 0, 3).reshape(nch * QC, D)


_NC = None


def kernel(**inputs):
    global _NC
    x = np.asarray(inputs["x"], np.float32)
    Wq = np.asarray(inputs["Wq"], np.float32)
    Wk = np.asarray(inputs["Wk"], np.float32)
    Wv = np.asarray(inputs["Wv"], np.float32)
    Wo = np.asarray(inputs["Wo"], np.float32)

    # If BASS_TRACE is set in the environment, run_bass_kernel_spmd tries to
    # import antenv.axon_hooks, which this image lacks; give it a null shim
    # so tracing degrades to a plain run instead of crashing.
    if "antenv.axon_hooks" not in sys.modules:
        try:
            import antenv.axon_hooks  # noqa: F401
        except ImportError:
            import types

            _shim = types.ModuleType("antenv.axon_hooks")
            _shim.get_axon_ntff_profile_hook = lambda: None
            _shim.set_axon_ntff_profile_hook = lambda h: None
            sys.modules["antenv.axon_hooks"] = _shim

    from concourse.bass_utils import run_bass_kernel_spmd

    if _NC is None:
        _NC = build_nc()
    in_maps = make_in_maps(x, Wq, Wk, Wv, Wo)
    res = run_bass_kernel_spmd(_NC, in_maps, core_ids=list(range(NCORES)))
    y = unshuffle_y(res.results[0]["y"]).astype(np.float32)
    for c in range(1, NCORES):
        y = y + unshuffle_y(res.results[c]["y"]).astype(np.float32)
    return y.reshape(B, T, D)


# revision 44
# speedup vs baseline: 1.1968x; 1.1968x over previous
"""Multi-head attention (B=4, T=2048, D=1024, H=16, causal) on 8 trn2 cores.

Sharding: 2 heads per core (tensor-parallel on H). Each core computes
q/k/v projections for its 128-row slice of Wq/Wk/Wv over all tokens,
causal attention for its 2 heads over all 4 batches, and a partial
o_proj contribution y_c = A_c @ Wo[:, slice].T.  The host sums the 8
partial outputs (the "all-reduce after o_proj" of the sharding hint).

Layout strategy: activations live transposed ([feature, token]) on
device so the matmul contraction dim is always the partition dim:
  qT/kT/vT [128=2*64, T]  <- W-slice-T tiles @ xT chunks
  scoresT  [128 k-tok, 512 q-tok] <- kT-tile.T @ qT   (per head)
  causal:  diagonal k-tiles trim scores/exp/attn@v to the valid q-span
           [off, QC) (no masked-region compute, no ep memsets); the
           intra-tile triangle is a 0/1 band multiply.
  softmax: no max-subtraction (logits are O(+-8); exp is safe in f32);
           exp on ACT; row sums come free as a column of ones appended
           to v; normalization deferred to after attn@v: broadcast the
           denominators across partitions with K=1 PE outer products,
           then reciprocal+scale in column HALVES so the first o_proj
           piece (needs only aT[:, 0:128]) starts after half the
           reciprocal latency.
  aT       [65, 512] PSUM accum over k-tiles (row 64 = softmax denom)
  y        [tok, D] bf16 partial via lhsT=aT tiles (halves the output
           DMA; the host sums the 8 partials in f32).
Matmuls run in bf16 (1 cyc/row on the PE; f32r measured 2 cyc/row).

DMA layouts are pre-shuffled on the host so every device DMA is
contiguous per partition (4KB+ descriptors): xT arrives as
[128, B, 8, T], weights as [128, 8, 128], y leaves as
[128, chunk, 4, D].  vT is bf16 end-to-end (bf16 PE transposes at
1 cyc/row, 2x DVE copy rate).  o_proj evacuation alternates
DVE/ACT and PSUM banks so the drain is double-buffered.
"""

import sys

sys.path.insert(0, "/opt/trn_rl_repo")

import ml_dtypes
import numpy as np

import concourse.bass as bass
import concourse.mybir as mybir
import concourse.tile as tile
from concourse.masks import make_identity

F32 = mybir.dt.float32
U32 = mybir.dt.uint32
F32R = mybir.dt.float32r
BF16 = mybir.dt.bfloat16
EXP = mybir.ActivationFunctionType.Exp

B, T, D, H = 4, 2048, 1024, 16
DH = D // H
NCORES = 8
HPC = H // NCORES          # heads per core (2)
HS = HPC * DH              # head-slice width per core (128)
QC = 512                   # q-tokens per chunk (PSUM free-dim limit, f32)
KT = 128                   # k-tokens per tile (partition dim)

_wsplit_n = [0]


def split_embedded_waits(nc):
    """Move embedded on_wait conditions into standalone EventSemaphore
    instructions.  The walrus build in this container rejects embedded
    sync waits on compute instruction structs ("Too many sync wait
    commands"); raw-bass-style standalone waits compile fine."""
    moved = 0
    for func in nc.m.functions:
        for blk in func.blocks:
            insts = list(blk.instructions)
            out = []
            changed = False
            for ins in insts:
                si = ins.sync_info
                waits = list(si.on_wait) if (si is not None and si.on_wait) else []
                limit = 1 if ins.opcode in ("EventSemaphore", "Drain") else 0
                if len(waits) > limit:
                    head = waits[:-limit] if limit else waits
                    tail = waits[-limit:] if limit else []
                    for w in head:
                        _wsplit_n[0] += 1
                        out.append(
                            mybir.InstEventSemaphore(
                                name=f"I-wsplit-{_wsplit_n[0]}",
                                engine=ins.engine,
                                sync_info=mybir.SyncInfo(on_wait=[w], on_update=[]),
                            )
                        )
                        moved += 1
                    ins.sync_info = mybir.SyncInfo(
                        on_wait=list(tail),
                        on_update=list(si.on_update) if si.on_update else [],
                    )
                    changed = True
                out.append(ins)
            if changed:
                blk.instructions = out
    return moved


def build_nc(nb=B, t=T, split_waits=True):
    """Build the per-core Bass program. nb/t shrinkable for simulation.
    split_waits must be True for hardware (walrus); False for CoreSim."""
    nqc = t // QC            # q-chunks per batch
    nkt = t // KT            # k-tiles per batch
    ntok = nb * t
    nd = D // 128            # 8 d-in tiles

    nc = bass.Bass("TRN2", target_bir_lowering=False)

    # pre-shuffled host layouts: every DMA is contiguous per partition
    xT_d = nc.dram_tensor("xT", [128, nb, nqc, nd, QC], BF16, kind="ExternalInput")
    wq_d = nc.dram_tensor("wq", [128, nd, HS], BF16, kind="ExternalInput")
    wk_d = nc.dram_tensor("wk", [128, nd, HS], BF16, kind="ExternalInput")
    wv_d = nc.dram_tensor("wv", [128, nd, HS], BF16, kind="ExternalInput")
    wo_d = nc.dram_tensor("wo", [HS, D], BF16, kind="ExternalInput")
    tri_d = nc.dram_tensor("tri", [KT, KT], BF16, kind="ExternalInput")
    y_d = nc.dram_tensor("y", [128, nb * nqc, QC // 128, D], BF16,
                         kind="ExternalOutput")

    with tile.TileContext(nc) as tc:
        with (
            tc.tile_pool(name="wpool", bufs=1) as wpool,
            tc.tile_pool(name="xin", bufs=2) as xin,
            tc.tile_pool(name="qkv", bufs=2) as qkvp,
            tc.tile_pool(name="vaug", bufs=2) as vaugp,
            tc.tile_pool(name="exps", bufs=4) as expp,
            tc.tile_pool(name="misc", bufs=2) as miscp,
            tc.tile_pool(name="yout", bufs=2) as youtp,
            tc.tile_pool(name="psc", bufs=2, space="PSUM") as psc,
            tc.tile_pool(name="pa", bufs=1, space="PSUM") as pa,
            tc.tile_pool(name="pm", bufs=2, space="PSUM") as pm,
        ):
            # resident weights (issue order = DMA arrival priority: the
            # first QKV matmul needs wq + the first x slice)
            wq_sb = wpool.tile([128, nd, HS], BF16, tag="wq")
            wk_sb = wpool.tile([128, nd, HS], BF16, tag="wk")
            wv_sb = wpool.tile([128, nd, HS], BF16, tag="wv")
            nc.sync.dma_start(wq_sb[:], wq_d[:])

            def load_xt(b):
                xt = xin.tile([128, nqc, nd, QC], BF16, tag="xt")
                nc.sync.dma_start(xt[:], xT_d[:, b])
                return xt

            def load_xt_split(b, c, xt=None):
                # one q-chunk of x: 8KB contiguous per partition
                if xt is None:
                    xt = xin.tile([128, nqc, nd, QC], BF16, tag="xt")
                nc.sync.dma_start(xt[:, c], xT_d[:, b, c])
                return xt

            # chunk 0 of batch 0 needs wq + x[0:512] + wk + wv, in that
            # order; the rest of batch 0's x can trail behind.
            xt_next = load_xt_split(0, 0)
            nc.sync.dma_start(wk_sb[:], wk_d[:])
            nc.sync.dma_start(wv_sb[:], wv_d[:])
            for s in range(1, nqc):
                load_xt_split(0, s, xt_next)
            wo_sb = wpool.tile([HS, D], BF16, tag="wo")
            nc.sync.dma_start(wo_sb[:], wo_d[:])
            tri_sb = wpool.tile([KT, KT], BF16, tag="tri")
            nc.sync.dma_start(tri_sb[:], tri_d[:])
            # constants: f32 masters, rounded into matmul dtypes via copies
            ones_f = wpool.tile([128, max(nkt, 128)], F32, tag="ones_f")
            nc.vector.memset(ones_f[:], 1.0)
            zeros_f = wpool.tile([1, 64], F32, tag="zeros_f")
            nc.vector.memset(zeros_f[:], 0.0)
            # head-expander rows: e0row = [1]*64+[0]*64, e1row = [0]*64+[1]*64
            e0row = wpool.tile([1, 128], F32R, tag="e0row")
            e1row = wpool.tile([1, 128], F32R, tag="e1row")
            nc.vector.tensor_copy(e0row[:, 0:64], ones_f[0:1, 0:64])
            nc.vector.tensor_copy(e0row[:, 64:128], zeros_f[:])
            nc.vector.tensor_copy(e1row[:, 0:64], zeros_f[:])
            nc.vector.tensor_copy(e1row[:, 64:128], ones_f[0:1, 0:64])
            ident = wpool.tile([128, 128], BF16, tag="ident")
            make_identity(nc, ident[:])
            ck = wpool.tile([128, 512], U32, tag="ck")
            nc.vector.memset(ck[:], 0x7EF127EA)
            # touch the ACT engine once now: walrus inserts the ~2.7us
            # activation-table load before the first ACTIVATE, and this
            # hides it under the startup DMA wait
            nc.scalar.copy(zeros_f[:], zeros_f[:])

            class OproJob:
                """Deferred o_proj for one 512-token chunk: 8 matmul+copy
                pieces stepped one at a time between k-iterations, then one
                fused DMA of the [512, D] result.  Pieces alternate PSUM
                banks (tags m/rb) and evacuation engines (DVE/ACT) so the
                matmul->copy chain is double-buffered even at drains."""

                def __init__(self, ci, aT, final=False):
                    self.ci, self.aT, self.final = ci, aT, final
                    self.ysb = youtp.tile([128, QC // 128, D], BF16, tag="ysb")
                    self.pieces = [
                        (tt, do)
                        for tt in range(QC // 128)
                        for do in range(D // 512)
                    ]
                    self.i = 0

                def step(self):
                    tt, do = self.pieces[self.i]
                    self.i += 1
                    if self.final and self.i % 2 == 0:
                        # the very last job has no following k-loop to
                        # drip through: double-buffer its drain via the
                        # psc banks (free once the last exps have run;
                        # the pm "rb" bank is NOT safe here -- the NR
                        # reciprocal still reads the denominators)
                        yp = psc.tile([128, 2, QC], F32, tag="sc", name="ypf")[:, 0, :]
                    else:
                        yp = pm.tile([128, 512], F32, tag="m", bufs=1,
                                     name="yp")
                    nc.tensor.matmul(
                        yp[:],
                        self.aT[:, 128 * tt : 128 * (tt + 1)],
                        wo_sb[:, 512 * do : 512 * (do + 1)],
                        start=True,
                        stop=True,
                    )
                    dst = self.ysb[:, tt, 512 * do : 512 * (do + 1)]
                    if self.final:
                        # ACT is idle in the endgame; keep the DVE queue
                        # clear for the half-1 normalization chain
                        nc.scalar.copy(dst, yp[:])
                    else:
                        nc.vector.tensor_copy(dst, yp[:])
                    if self.final and self.i % 2 == 0:
                        # nothing left to overlap the last DMA with: ship
                        # each finished 128-token block immediately
                        nc.sync.dma_start(
                            y_d[:, self.ci, tt, :], self.ysb[:, tt, :]
                        )
                    elif not self.final and self.i == len(self.pieces):
                        nc.sync.dma_start(y_d[:, self.ci, :, :], self.ysb[:])
                    return self.i < len(self.pieces)

            ojob = [None]

            # kT0z/kT1z zero halves never change: single-buffered tiles,
            # memset once (batches are strictly sequential on this layout)
            kT0z = qkvp.tile([128, t], BF16, tag="kT0z", bufs=1, name="kT0z")
            kT1z = qkvp.tile([128, t], BF16, tag="kT1z", bufs=1, name="kT1z")
            nc.vector.memset(kT0z[64:128, :], 0.0)
            nc.vector.memset(kT1z[0:64, :], 0.0)

            qnext = [None]  # next batch's qT, chunk 0 pre-projected
            for b in range(nb):
                xt_b = xt_next
                # ---- q/k/v projections for batch b ----
                # kT is stored zero-padded per head (kT0z rows 0:64 = head0,
                # rows 64:128 = 0; kT1z the reverse) so the scores matmul can
                # stream the full 128-partition qT at full SBUF rate.
                if qnext[0] is not None:
                    qT, q_pre = qnext[0], True
                    qnext[0] = None
                else:
                    qT, q_pre = qkvp.tile([128, t], BF16, tag="qT", name="qT"), False
                vT = qkvp.tile([128, t], BF16, tag="vT")
                for ch in range(nqc):
                    cs = slice(QC * ch, QC * (ch + 1))
                    for wi, ws in enumerate((wq_sb, wk_sb, wv_sb)):
                        if ch == 0 and wi == 0 and q_pre:
                            continue  # hoisted into the previous batch tail
                        # the first projection of a batch runs on the free
                        # pm bank: at a batch boundary both psc tiles are
                        # still being read by the previous batch's tail
                        # exps, which would idle the PE (and HAM-throttle
                        # it); by the time the second projection starts
                        # the first exp has freed its psc buffer.
                        if ch == 0 and wi == 0:
                            ps = pm.tile([128, QC], F32, tag="m", bufs=1,
                                         name="ps")[:, :]
                        else:
                            ps = psc.tile([128, 2, QC], F32, tag="sc", name="psq")[:, 0, :]
                        for kd in range(nd):
                            nc.tensor.matmul(
                                ps,
                                ws[:, kd, :],
                                xt_b[:, ch, kd, :],
                                start=(kd == 0),
                                stop=(kd == nd - 1),
                            )
                        # evacuate on the Act engine: it is idle during
                        # the qkv phase while DVE is the congested queue
                        if wi == 0:
                            nc.scalar.copy(qT[:, cs], ps)
                        elif wi == 1:
                            nc.scalar.copy(kT0z[0:64, cs], ps[0:64, :])
                            nc.scalar.copy(kT1z[64:128, cs], ps[64:128, :])
                        else:
                            nc.scalar.copy(vT[:, cs], ps[:, :])

                if b + 1 < nb:
                    xt_next = load_xt(b + 1)

                # ---- transpose v into [k-tok, dh(+ones)] tiles ----
                v0 = vaugp.tile([128, nkt, DH + 1], BF16, tag="v0")
                v1 = vaugp.tile([128, nkt, DH + 1], BF16, tag="v1")
                nc.vector.tensor_copy(v0[:, :, DH : DH + 1], ones_f[:, 0:nkt])
                nc.vector.tensor_copy(v1[:, :, DH : DH + 1], ones_f[:, 0:nkt])
                for kt0 in range(0, nkt, 4):
                    # four transposes back-to-back into one PSUM tile, then
                    # two strided copies: avoids the per-tile PE<->DVE
                    # ping-pong through the single rb bank
                    ng = min(4, nkt - kt0)
                    tp = pm.tile([128, 512], BF16, tag="rb", bufs=1, name="tp")
                    for j in range(ng):
                        kt = kt0 + j
                        nc.tensor.transpose(
                            tp[:, 128 * j : 128 * (j + 1)],
                            vT[:, KT * kt : KT * (kt + 1)],
                            ident[:],
                        )
                    tpv = tp.rearrange("p (g c) -> p g c", c=128)
                    nc.vector.tensor_copy(
                        v0[:, kt0 : kt0 + ng, 0:DH], tpv[:, 0:ng, 0:DH]
                    )
                    nc.vector.tensor_copy(
                        v1[:, kt0 : kt0 + ng, 0:DH], tpv[:, 0:ng, DH : 2 * DH]
                    )

                # ---- attention + o_proj per q-chunk ----
                # k-loop emitted software-pipelined (scores two steps ahead
                # of attn@v); the previous chunk's o_proj matmuls and output
                # copies are drip-fed between k-iterations so the PE never
                # sits in a blocked o_proj stretch, and normalization uses a
                # magic-seed Newton-Raphson reciprocal on DVE.
                carried = [None]  # pre-emitted scores for (qc+1, kt=0)
                for qc in range(nqc):
                    q0 = QC * qc
                    apair = pa.tile([DH + 1, 2, QC], F32, tag="apair")
                    hi = qc * (QC // KT) + (QC // KT)  # causal: k-tiles 0..hi-1

                    def emit_scores(kt, sq0=None):
                        sq0 = q0 if sq0 is None else sq0
                        o = max(KT * kt - sq0, 0)
                        scp = psc.tile([128, 2, QC], F32, tag="sc")
                        nc.tensor.matmul(
                            scp[:, 0, o:QC],
                            kT0z[:, KT * kt : KT * (kt + 1)],
                            qT[:, sq0 + o : sq0 + QC],
                            start=True,
                            stop=True,
                        )
                        nc.tensor.matmul(
                            scp[:, 1, o:QC],
                            kT1z[:, KT * kt : KT * (kt + 1)],
                            qT[:, sq0 + o : sq0 + QC],
                            start=True,
                            stop=True,
                        )
                        return scp

                    def emit_tail(kt, scp, tri_eng=None):
                        ep = expp.tile([128, 2, QC], BF16, tag="ep")
                        off = KT * kt - q0
                        o = max(off, 0)
                        nc.scalar.activation(ep[:, :, o:QC], scp[:, :, o:QC], EXP)
                        if off >= 0:
                            # diagonal tile: apply the 0/1 band (no memset
                            # needed -- attn@v only reads the [o:QC] span)
                            for h in (0, 1):
                                (tri_eng or nc.vector).tensor_mul(
                                    ep[:, h, o : o + KT],
                                    ep[:, h, o : o + KT],
                                    tri_sb[:],
                                )
                        for h, vh in ((0, v0), (1, v1)):
                            nc.tensor.matmul(
                                apair[:, h, o:QC],
                                vh[:, kt, :],
                                ep[:, h, o:QC],
                                start=(kt == 0),
                                stop=(kt == hi - 1),
                                skip_group_check=True,
                            )

                    if carried[0] is not None:
                        pend = [carried[0]]
                        carried[0] = None
                    else:
                        pend = [emit_scores(0)]
                    if hi > 1:
                        pend.append(emit_scores(1))
                    for kt in range(2, hi):
                        emit_tail(kt - 2, pend.pop(0))
                        pend.append(emit_scores(kt))
                        if ojob[0] is not None and not ojob[0].step():
                            ojob[0] = None
                    final = b == nb - 1 and qc == nqc - 1
                    if final:
                        # ===== custom endgame for the very last chunk =====
                        # There is no later work to hide the normalization
                        # chain behind, so exploit causality: the last two
                        # (diagonal) k-tiles only touch apair columns
                        # [256:512], so columns [0:256] are already final.
                        # Evacuate and normalize half 0 on DVE *while* the
                        # PE runs the last two tails, then drain o_proj
                        # pieces per half.  Tri-masks go to the idle GpSimd
                        # so they don't block the DVE chain (FIFO queues).
                        H1 = QC // 2
                        sums01 = miscp.tile([1, 2, QC], F32R, tag="sums01")
                        aT = qkvp.tile([128, QC], BF16, tag="aT", bufs=3)
                        nc.vector.tensor_copy(
                            sums01[:, :, 0:H1], apair[DH : DH + 1, :, 0:H1]
                        )
                        nc.vector.tensor_copy(
                            aT[0:DH, 0:H1], apair[0:DH, 0, 0:H1]
                        )
                        nc.vector.tensor_copy(
                            aT[DH : 2 * DH, 0:H1], apair[0:DH, 1, 0:H1]
                        )
                        for j, scp in enumerate(pend):
                            emit_tail(
                                hi - len(pend) + j, scp, tri_eng=nc.gpsimd
                            )
                        rb = pm.tile([128, 512], F32, tag="rb", bufs=1)
                        rcp = miscp.tile([128, QC], F32, tag="rcp")
                        tnr = miscp.tile([128, QC], F32, tag="tnr")
                        rb_f = rb[:, 0:QC]
                        job = OproJob(b * nqc + qc, aT, final=True)

                        def warm():
                            # no-op weight load: keeps the PE active
                            # through the endgame's short waits so HAM
                            # doesn't down-clock the whole tail
                            nc.tensor.ldweights(wo_sb[:, 0:128])

                        for half in range(2):
                            hs = slice(half * H1, (half + 1) * H1)
                            if half == 1:
                                nc.vector.tensor_copy(
                                    sums01[:, :, hs],
                                    apair[DH : DH + 1, :, hs],
                                )
                                nc.vector.tensor_copy(
                                    aT[0:DH, hs], apair[0:DH, 0, hs]
                                )
                                nc.vector.tensor_copy(
                                    aT[DH : 2 * DH, hs], apair[0:DH, 1, hs]
                                )
                            nc.tensor.matmul(
                                rb[:, hs], e0row[:], sums01[:, 0, hs],
                                start=True, stop=False, skip_group_check=True,
                            )
                            nc.tensor.matmul(
                                rb[:, hs], e1row[:], sums01[:, 1, hs],
                                start=False, stop=True, skip_group_check=True,
                            )
                            nc.vector.tensor_tensor(
                                rcp[:, hs].bitcast(U32),
                                ck[:, hs],
                                rb_f[:, hs].bitcast(U32),
                                mybir.AluOpType.subtract,
                            )
                            nc.vector.tensor_mul(
                                tnr[:, hs], rb_f[:, hs], rcp[:, hs]
                            )
                            nc.vector.scalar_tensor_tensor(
                                rcp[:, hs], tnr[:, hs], 2.0, rcp[:, hs],
                                mybir.AluOpType.subtract, mybir.AluOpType.mult,
                            )
                            nc.vector.tensor_mul(
                                aT[:, hs], aT[:, hs], rcp[:, hs]
                            )
                            warm()
                            for _ in range(4):
                                job.step()
                                warm()
                        continue
                    for j, scp in enumerate(pend):
                        emit_tail(hi - len(pend) + j, scp)
                    # bridge the chunk boundary: pre-emit the next chunk's
                    # first scores pair while the norm copies drain
                    if qc + 1 < nqc:
                        carried[0] = emit_scores(0, sq0=QC * (qc + 1))
                    while ojob[0] is not None:
                        if not ojob[0].step():
                            ojob[0] = None

                    if qc == nqc - 1 and b + 1 < nb:
                        # batch boundary: fill the PE during this chunk's
                        # normalization with the NEXT batch's first
                        # q-projection (pm bank, x already resident)
                        qTn = qkvp.tile([128, t], BF16, tag="qT", name="qTn")
                        psn = pm.tile([128, QC], F32, tag="m", bufs=1,
                                      name="psn")
                        for kd in range(nd):
                            nc.tensor.matmul(
                                psn[:, :],
                                wq_sb[:, kd, :],
                                xt_next[:, 0, kd, :],
                                start=(kd == 0),
                                stop=(kd == nd - 1),
                            )
                        nc.scalar.copy(qTn[:, 0:QC], psn[:, :])
                        qnext[0] = qTn

                    # free apair fast: pull out the two heads + denominators
                    sums01 = miscp.tile([1, 2, QC], F32R, tag="sums01")
                    nc.vector.tensor_copy(sums01[:], apair[DH : DH + 1, :, :])
                    aT = qkvp.tile([128, QC], BF16, tag="aT", bufs=3)
                    nc.scalar.copy(aT[0:DH, :], apair[0:DH, 0, :])
                    nc.scalar.copy(aT[DH : 2 * DH, :], apair[0:DH, 1, :])
                    # normalization, fully pipelined by column halves: copy
                    # the denominator row out (lane-serial, the long pole),
                    # broadcast it across partitions (K=1 PE outer
                    # products), 1/s via magic-seed + one Newton-Raphson
                    # pass on DVE ([s_bits XOR ~0] + [K+1] seed in one
                    # fused op; the (t-2)*r0 combine yields -1/s, absorbed
                    # by staging -Wo), then scale aT.  Doing all five steps
                    # per half instead of splitting only the NR shortens
                    # the last-attnv -> first-o_proj critical path by ~1us.
                    rb = pm.tile([128, 512], F32, tag="rb", bufs=1)
                    nc.tensor.matmul(
                        rb[:, 0:QC], e0row[:], sums01[:, 0, :],
                        start=True, stop=False, skip_group_check=True,
                    )
                    nc.tensor.matmul(
                        rb[:, 0:QC], e1row[:], sums01[:, 1, :],
                        start=False, stop=True, skip_group_check=True,
                    )
                    rcp = miscp.tile([128, QC], F32, tag="rcp")
                    tnr = miscp.tile([128, QC], F32, tag="tnr")
                    rb_f = rb[:, 0:QC]
                    for half in range(2):
                        hs = slice(half * (QC // 2), (half + 1) * (QC // 2))
                        nc.vector.tensor_tensor(
                            rcp[:, hs].bitcast(U32),
                            ck[:, hs],
                            rb_f[:, hs].bitcast(U32),
                            mybir.AluOpType.subtract,
                        )
                        nc.vector.tensor_mul(tnr[:, hs], rb_f[:, hs], rcp[:, hs])
                        nc.vector.scalar_tensor_tensor(
                            rcp[:, hs], tnr[:, hs], 2.0, rcp[:, hs],
                            mybir.AluOpType.subtract, mybir.AluOpType.mult,
                        )
                        nc.vector.tensor_mul(aT[:, hs], aT[:, hs], rcp[:, hs])
                    ojob[0] = OproJob(b * nqc + qc, aT)
            while ojob[0] is not None:
                if not ojob[0].step():
                    ojob[0] = None

    if split_waits:
        split_embedded_waits(nc)
    return nc


def make_tri():
    tri = np.zeros((KT, KT), np.float32)
    j = np.arange(KT)[None, :]
    k = np.arange(KT)[:, None]
    tri[j >= k] = 1.0
    return tri.astype(ml_dtypes.bfloat16)


def shuffle_w(w):
    # [D, HS] -> [128, nd, HS]: row a*128+p lands at [p, a, :]
    return np.ascontiguousarray(
        w.reshape(D // 128, 128, HS).transpose(1, 0, 2)
    )


def make_in_maps(x, Wq, Wk, Wv, Wo):
    bf = ml_dtypes.bfloat16
    # x [B, T, D] -> [128, B, nqc, 8, QC]:
    # xbuf[p, b, c, a, tc] = x[b, c*QC+tc, a*128+p]  (chunk-contiguous:
    # each q-chunk is one 8KB run per partition)
    xT = np.ascontiguousarray(
        x.reshape(B, T // QC, QC, D // 128, 128).transpose(4, 0, 1, 3, 2)
    ).astype(bf)
    tri = make_tri()
    scale = np.float32(1.0 / np.sqrt(DH))
    in_maps = []
    for c in range(NCORES):
        hs = slice(HS * c, HS * (c + 1))
        in_maps.append(
            {
                "xT": xT,
                "wq": shuffle_w((Wq[hs, :] * scale).T.astype(bf)),
                "wk": shuffle_w(Wk[hs, :].T.astype(bf)),
                "wv": shuffle_w(Wv[hs, :].T.astype(bf)),
                "wo": np.ascontiguousarray(-Wo[:, hs].T).astype(bf),
                "tri": tri,
            }
        )
    return in_maps


def unshuffle_y(y):
    # y [128, nchunks, 4, D] -> [ntok, D]: token c*512 + a*128 + p
    nch = y.shape[1]
    return y.transpose(1, 2, 0, 3).reshape(nch * QC, D)


_NC = None


def kernel(**inputs):
    global _NC
    x = np.asarray(inputs["x"], np.float32)
    Wq = np.asarray(inputs["Wq"], np.float32)
    Wk = np.asarray(inputs["Wk"], np.float32)
    Wv = np.asarray(inputs["Wv"], np.float32)
    Wo = np.asarray(inputs["Wo"], np.float32)

    # If BASS_TRACE is set in the environment, run_bass_kernel_spmd tries to
    # import antenv.axon_hooks, which this image lacks; give it a null shim
    # so tracing degrades to a plain run instead of crashing.
    if "antenv.axon_hooks" not in sys.modules:
        try:
            import antenv.axon_hooks  # noqa: F401
        except ImportError:
            import types

            _shim = types.ModuleType("antenv.axon_hooks")
            _shim.get_axon_ntff_profile_hook = lambda: None
            _shim.set_axon_ntff_profile_hook = lambda h: None
            sys.modules["antenv.axon_hooks"] = _shim

    from concourse.bass_utils import run_bass_kernel_spmd

    if _NC is None:
        _NC = build_nc()
    in_maps = make_in_maps(x, Wq, Wk, Wv, Wo)
    res = run_bass_kernel_spmd(_NC, in_maps, core_ids=list(range(NCORES)))
    y = unshuffle_y(res.results[0]["y"]).astype(np.float32)
    for c in range(1, NCORES):
        y = y + unshuffle_y(res.results[c]["y"]).astype(np.float32)
    return y.reshape(B, T, D)
